# revision 3
# baseline (speedup 1.0000x reference)
"""Causal GQA attention block (QK L2-norm + RoPE) for 8 trn2 NeuronCores.

Sharding: tensor-parallel over head-halves (2) x data-parallel over batch (4).
Core c handles batch c//2 and heads [h*8, h*8+8) with h = c%2.

Fast-path design (vs the fp32r baseline):
  - QK projection runs in fp8e4m3 with DoubleRow perf mode (256-deep
    contraction per pass = 2x PE throughput).  Weights are pre-scaled by
    64 so fp8 covers their range; the L2 norm absorbs the scale exactly.
  - Linear softmax: with QK-norm the logits are bounded by +-0.0884, so
    exp(p) is replaced by 1 + p (error ~p^2/2 < 0.4%).  att = 1 + SCALE*s
    splits attention into an exact "past-sum" base (bf16 matmuls against
    host triangle tables + per-qtile column sums of V) plus a correction
    d8 (x) v8 where d8 = fp8(raw scores) - both the correction AV and the
    softmax row-sum run as fp8 DoubleRow matmuls at 2x rate.
  - V path / scores / w_o in bf16; Q stays SBUF-resident (no DRAM spill).
  - Finalize is one fused DVE op: o = (o_ps + Sv) * recip(iota + sum).
All scale factors are folded into host tables (lambda = 1/SCALE).
"""

import numpy as np
import ml_dtypes

import concourse.mybir as mybir
import concourse.tile as tile
from concourse import bacc
from concourse import bass2jax

F32 = mybir.dt.float32
F32R = mybir.dt.float32r
BF16 = mybir.dt.bfloat16
F8 = mybir.dt.float8e4
AF = mybir.ActivationFunctionType
ALU = mybir.AluOpType
PM = mybir.MatmulPerfMode

NPF8 = ml_dtypes.float8_e4m3
NPBF = ml_dtypes.bfloat16

P = 128
B, T, D = 4, 2048, 2048
N_HEADS, HEAD_DIM, N_KV = 16, 128, 4
Q_DIM = N_HEADS * HEAD_DIM          # 2048
KV_DIM = N_KV * HEAD_DIM            # 512
H_Q = 8                             # q heads per core
H_KV = 2                            # kv heads per core
EQ = H_Q * HEAD_DIM                 # 1024 q features per core
EKV = H_KV * HEAD_DIM               # 256
SCALE = 0.08838834764831845
LAM = 1.0 / SCALE
WSCALE = 64.0                       # fp8 pre-scale on w_qk (norm absorbs it)
THETA = 10000.0

KSUB = D // P                       # 16 contraction subtiles
N_CORES = 8
TT_HALF = T // 2                    # 1024, phase-1 token half
NT512 = T // 512                    # 4 512-token q tiles
NTB = T // P                        # 16 128-token blocks


def _build_module():
    nc = bacc.Bacc("TRN2", target_bir_lowering=False, debug=False)

    x16t = nc.dram_tensor("x16t", [D, T], BF16, kind="ExternalInput")
    x8t = nc.dram_tensor("x8t", [D, T], F8, kind="ExternalInput")
    wq8 = nc.dram_tensor("wq8", [H_Q, P, KSUB, P], F8, kind="ExternalInput")
    wk8 = nc.dram_tensor("wk8", [P, KSUB, EKV], F8, kind="ExternalInput")
    wv16 = nc.dram_tensor("wv16", [P, KSUB, EKV], BF16, kind="ExternalInput")
    wo16 = nc.dram_tensor("wo16", [P, H_Q, D], BF16, kind="ExternalInput")
    cos_t = nc.dram_tensor("cos_t", [P, T], BF16, kind="ExternalInput")
    sin_t = nc.dram_tensor("sin_t", [P, T], BF16, kind="ExternalInput")
    ones_m = nc.dram_tensor("ones_m", [P, P], F32R, kind="ExternalInput")
    pswap = nc.dram_tensor("pswap", [P, P], F32R, kind="ExternalInput")
    ones8 = nc.dram_tensor("ones8", [P, 2, P], F8, kind="ExternalInput")
    tri_t = nc.dram_tensor("tri_t", [P, 4, 512], BF16, kind="ExternalInput")
    mask_t = nc.dram_tensor("mask_t", [P, 2, 2, 512], F32R, kind="ExternalInput")
    iota_t = nc.dram_tensor("iota_t", [P, T], F32, kind="ExternalInput")
    lam_t = nc.dram_tensor("lam_t", [P, KSUB, 4], BF16, kind="ExternalInput")
    out_t = nc.dram_tensor("out_t", [D, T], F32, kind="ExternalOutput")

    with tile.TileContext(nc) as tc:
        with (
            tc.tile_pool(name="persist", bufs=1) as persist,
            tc.tile_pool(name="kv_persist", bufs=1) as kvp,
            tc.tile_pool(name="att_sb", bufs=5) as att_sb,
        ):
            ones_sb = persist.tile([P, P], F32R)
            psw_sb = persist.tile([P, P], F32R)
            ones8_sb = persist.tile([P, 2, P], F8)
            tri_sb = persist.tile([P, 4, 512], BF16)
            mask_sb = persist.tile([P, 2, 2, 512], F32R)
            iota_sb = persist.tile([P, T], F32)
            lam_sb = persist.tile([P, KSUB, 4], BF16)
            sv_sb = persist.tile([P, 2, 4], F32R)
            nc.sync.dma_start(ones_sb[:], ones_m.ap())
            nc.sync.dma_start(psw_sb[:], pswap.ap())
            nc.sync.dma_start(ones8_sb[:], ones8.ap())
            nc.gpsimd.dma_start(tri_sb[:], tri_t.ap())
            nc.gpsimd.dma_start(mask_sb[:], mask_t.ap())
            nc.gpsimd.dma_start(iota_sb[:], iota_t.ap())
            nc.gpsimd.dma_start(lam_sb[:], lam_t.ap())
            k_sb = kvp.tile([P, H_KV, T], BF16)     # roped+normed K^T slabs
            v16_sb = kvp.tile([P, NTB, EKV], BF16)  # V in [t, e] layout
            v8_sb = kvp.tile([P, NTB, EKV], F8)
            q_all = kvp.tile([P, H_Q, T], BF16)     # Q resident in SBUF

            # ---------------- phase 1: qkv proj + L2 norm + rope ----------
            with (
                tc.tile_pool(name="xres", bufs=1) as xres,
                tc.tile_pool(name="wstream", bufs=2) as wstream,
                tc.tile_pool(name="wvres", bufs=1) as wvres,
                tc.tile_pool(name="p1tmp", bufs=2) as p1tmp,
                tc.tile_pool(name="trig", bufs=1) as trig,
                tc.tile_pool(name="pp", bufs=2, space="PSUM") as pp,
                tc.tile_pool(name="pssq", bufs=2, space="PSUM") as pssq,
                tc.tile_pool(name="psw", bufs=2, space="PSUM") as psw,
                tc.tile_pool(name="pv", bufs=2, space="PSUM") as pv,
            ):
                cos_sb = trig.tile([P, T], BF16)
                sin_sb = trig.tile([P, T], BF16)
                wv_sb = wvres.tile([P, KSUB, EKV], BF16)
                wk_sb = wvres.tile([P, KSUB, EKV], F8, name="wk_sb")
                nc.sync.dma_start(wk_sb[:], wk8.ap())
                for th in range(2):
                    t0 = th * TT_HALF
                    x16_sb = xres.tile([P, KSUB, TT_HALF], BF16, tag="x16")
                    x8_sb = xres.tile([P, KSUB, TT_HALF], F8, tag="x8")
                    xr16 = x16t.ap()[:, t0 : t0 + TT_HALF].rearrange(
                        "(ks p) t -> p ks t", p=P
                    )
                    xr8 = x8t.ap()[:, t0 : t0 + TT_HALF].rearrange(
                        "(ks p) t -> p ks t", p=P
                    )
                    for ks in range(KSUB):
                        nc.sync.dma_start(x8_sb[:, ks], xr8[:, ks])
                    for ks in range(KSUB):
                        nc.sync.dma_start(x16_sb[:, ks], xr16[:, ks])
                    if th == 0:
                        nc.sync.dma_start(cos_sb[:], cos_t.ap())
                        nc.sync.dma_start(sin_sb[:], sin_t.ap())
                        nc.sync.dma_start(wv_sb[:], wv16.ap())

                    def proj_norm_rope(es):
                        """project feature block es (fp8 DoubleRow), norm, rope"""
                        if es < H_Q:
                            w_sb = wstream.tile([P, KSUB, P], F8, tag="w")
                            nc.sync.dma_start(w_sb[:], wq8.ap()[es])
                            w_use = w_sb
                        else:
                            w_use = wk_sb
                        for tt in range(2):
                            tg = t0 + tt * 512
                            sl = slice(tt * 512, (tt + 1) * 512)
                            raw_ps = pp.tile([P, 512], F32, tag="raw")
                            for kp in range(KSUB // 2):
                                if es < H_Q:
                                    lhs = w_use[:, 2 * kp : 2 * kp + 2, :]
                                else:
                                    e0 = (es - H_Q) * P
                                    lhs = w_use[:, 2 * kp : 2 * kp + 2, e0 : e0 + P]
                                nc.tensor.matmul(
                                    raw_ps[:],
                                    lhs,
                                    x8_sb[:, 2 * kp : 2 * kp + 2, sl],
                                    start=(kp == 0),
                                    stop=(kp == KSUB // 2 - 1),
                                    perf_mode=PM.DoubleRow,
                                )
                            sq = p1tmp.tile([P, 512], F32R, tag="t1")
                            nc.scalar.activation(sq[:], raw_ps[:], AF.Square)
                            ssq_ps = pssq.tile([P, 512], F32, tag="ssq")
                            nc.tensor.matmul(
                                ssq_ps[:], ones_sb[:], sq[:], start=True, stop=True
                            )
                            s_sb = p1tmp.tile([P, 512], F32, tag="t2")
                            nc.scalar.activation(s_sb[:], ssq_ps[:], AF.Sqrt)
                            r_sb = p1tmp.tile([P, 512], F32, tag="t3")
                            nc.vector.reciprocal_approx_fast(r_sb[:], s_sb[:])
                            qn = p1tmp.tile([P, 512], F32R, tag="t4")
                            nc.vector.tensor_mul(qn[:], raw_ps[:], r_sb[:])
                            ys = p1tmp.tile([P, 512], F32R, tag="t1")
                            nc.vector.tensor_mul(
                                ys[:], qn[:], sin_sb[:, tg : tg + 512]
                            )
                            sw_ps = psw.tile([P, 512], F32, tag="sw")
                            nc.tensor.matmul(
                                sw_ps[:], psw_sb[:], ys[:], start=True, stop=True
                            )
                            qc = p1tmp.tile([P, 512], F32, tag="t2")
                            nc.vector.tensor_mul(
                                qc[:], qn[:], cos_sb[:, tg : tg + 512]
                            )
                            if es < H_Q:
                                nc.vector.tensor_add(
                                    q_all[:, es, tg : tg + 512], sw_ps[:], qc[:]
                                )
                            else:
                                nc.vector.tensor_add(
                                    k_sb[:, es - H_Q, tg : tg + 512],
                                    sw_ps[:],
                                    qc[:],
                                )

                    # K first so attention can start earliest, then Q, then V
                    for es in (H_Q, H_Q + 1):
                        proj_norm_rope(es)
                    for es in range(H_Q):
                        proj_norm_rope(es)
                    for tb in range(TT_HALF // P):
                        tbg = th * (TT_HALF // P) + tb
                        v_ps = pv.tile([P, EKV], F32, tag="vp")
                        for ks in range(KSUB):
                            nc.tensor.matmul(
                                v_ps[:],
                                x16_sb[:, ks, tb * P : (tb + 1) * P],
                                wv_sb[:, ks],
                                start=(ks == 0),
                                stop=(ks == KSUB - 1),
                            )
                        nc.scalar.copy(v16_sb[:, tbg], v_ps[:])
                        nc.scalar.copy(v8_sb[:, tbg], v_ps[:])

            # ------- phase 2: attention + output projection per q-tile ----
            with (
                tc.tile_pool(name="wores", bufs=1) as wores,
                tc.tile_pool(name="p2tmp", bufs=2) as p2tmp,
                tc.tile_pool(name="oall", bufs=2) as oall,
                tc.tile_pool(name="fout", bufs=3) as fout,
                tc.tile_pool(name="psc", bufs=2, space="PSUM") as psc,
                tc.tile_pool(name="pav", bufs=1, space="PSUM") as pav,
                tc.tile_pool(name="psum2", bufs=1, space="PSUM") as psum2,
                tc.tile_pool(name="pf", bufs=2, space="PSUM") as pf,
            ):
                wo_sb = wores.tile([P, H_Q, D], BF16)
                for ei in range(H_Q):
                    nc.gpsimd.dma_start(wo_sb[:, ei], wo16.ap()[:, ei])
                # cumulative column sums of V (lambda-scaled) per (kv, qt)
                for kvi in range(H_KV):
                    sv_ps = pav.tile([P, 4], F32, tag="av")
                    for kb in range(NTB):
                        nc.tensor.matmul(
                            sv_ps[:],
                            v16_sb[:, kb, kvi * P : (kvi + 1) * P],
                            lam_sb[:, kb],
                            start=(kb == 0),
                            stop=(kb == NTB - 1),
                        )
                    nc.scalar.copy(sv_sb[:, kvi], sv_ps[:])
                for qt in range(NT512):
                    q0 = qt * 512
                    nkb = (qt + 1) * 4
                    npair = nkb // 2
                    o_all = oall.tile([P, H_Q, 512], BF16, tag="oa")
                    for hd in range(H_Q):
                        kvi = hd // 4
                        d8s = []
                        for pj in range(npair):
                            kb0 = 2 * pj
                            sc_ps = psc.tile([P, 2, 512], F32, tag="sc")
                            for j in range(2):
                                kb = kb0 + j
                                nc.tensor.matmul(
                                    sc_ps[:, j],
                                    k_sb[:, kvi, kb * P : (kb + 1) * P],
                                    q_all[:, hd, q0 : q0 + 512],
                                    start=True,
                                    stop=True,
                                )
                            if pj >= npair - 2:
                                # diagonal pair: zero future keys before cast
                                pat = pj - (npair - 2)
                                nc.vector.tensor_mul(
                                    sc_ps[:], sc_ps[:], mask_sb[:, pat]
                                )
                            d8 = att_sb.tile([P, 2, 512], F8, tag="att")
                            nc.scalar.activation(d8[:], sc_ps[:], AF.Copy)
                            d8s.append(d8)
                        # o_ps = sum_k d8*v8 (DoubleRow) + lambda * past-sum V
                        o_ps = pav.tile([P, 512], F32, tag="av")
                        for pj, d8 in enumerate(d8s):
                            nc.tensor.matmul(
                                o_ps[:],
                                v8_sb[:, 2 * pj : 2 * pj + 2, kvi * P : (kvi + 1) * P],
                                d8[:],
                                start=(pj == 0),
                                stop=False,
                                perf_mode=PM.DoubleRow,
                                skip_group_check=True,
                            )
                        for r in range(4):
                            kb = nkb - 4 + r
                            nc.tensor.matmul(
                                o_ps[:, 128 * r :],
                                v16_sb[:, kb, kvi * P : (kvi + 1) * P],
                                tri_sb[:, r, 128 * r :],
                                start=False,
                                stop=(r == 3),
                                skip_group_check=True,
                            )
                        s_ps = psum2.tile([P, 512], F32, tag="sum")
                        for pj, d8 in enumerate(d8s):
                            nc.tensor.matmul(
                                s_ps[:],
                                ones8_sb[:],
                                d8[:],
                                start=(pj == 0),
                                stop=(pj == npair - 1),
                                perf_mode=PM.DoubleRow,
                            )
                        den = p2tmp.tile([P, 512], F32, tag="den")
                        nc.vector.tensor_add(
                            den[:], s_ps[:], iota_sb[:, q0 : q0 + 512]
                        )
                        rs = p2tmp.tile([P, 512], F32, tag="rs")
                        nc.vector.reciprocal_approx_fast(rs[:], den[:])
                        nc.vector.scalar_tensor_tensor(
                            o_all[:, hd],
                            o_ps[:],
                            sv_sb[:, kvi, qt : qt + 1],
                            rs[:],
                            op0=ALU.add,
                            op1=ALU.mult,
                        )
                    for eo in range(D // P):
                        f_ps = pf.tile([P, 512], F32, tag="f")
                        for ei in range(H_Q):
                            nc.tensor.matmul(
                                f_ps[:],
                                wo_sb[:, ei, eo * P : (eo + 1) * P],
                                o_all[:, ei],
                                start=(ei == 0),
                                stop=(ei == H_Q - 1),
                            )
                        f_sb = fout.tile([P, 512], F32, tag="fo")
                        nc.scalar.copy(f_sb[:], f_ps[:])
                        nc.sync.dma_start(
                            out_t.ap()[eo * P : (eo + 1) * P, q0 : q0 + 512],
                            f_sb[:],
                        )

    nc.compile()
    return nc


def _re3(a):
    """[K, E] -> [P, K//P, E] host rearrange for contiguous weight DMAs."""
    return np.ascontiguousarray(a.reshape(-1, P, a.shape[1]).transpose(1, 0, 2))


def _host_inputs(x, w_qkv, w_o):
    """Build the 8 per-core input maps from full inputs."""
    x = np.asarray(x, dtype=np.float32)
    w_qkv = np.asarray(w_qkv, dtype=np.float32)
    w_o = np.asarray(w_o, dtype=np.float32)

    half = HEAD_DIM // 2
    inv_freq = 1.0 / (
        THETA ** (np.arange(0, HEAD_DIM, 2, dtype=np.float32) / HEAD_DIM)
    )
    ang = np.arange(T, dtype=np.float32)[:, None] * inv_freq[None, :]  # [T, 64]
    cos = np.cos(ang).T.astype(np.float32)  # [64, T]
    sin = np.sin(ang).T.astype(np.float32)
    cos_t = np.ascontiguousarray(np.concatenate([cos, cos], axis=0)).astype(NPBF)
    sin_t = np.ascontiguousarray(np.concatenate([sin, sin], axis=0)).astype(NPBF)

    ones_m = np.ones((P, P), dtype=np.float32)
    pswap = np.zeros((P, P), dtype=np.float32)
    for p in range(half):
        pswap[p, p + half] = 1.0    # out[m=p+64] += ys[p]
        pswap[p + half, p] = -1.0   # out[m=p]    -= ys[p+64]
    ones8 = np.ones((P, 2, P), dtype=np.float32).astype(NPF8)

    t_idx = np.arange(P, dtype=np.float32)[:, None]        # key within block
    j_idx = np.arange(512, dtype=np.float32)[None, :]      # query col
    tri_t = np.zeros((P, 4, 512), dtype=np.float32)
    for r in range(4):
        tri_t[:, r] = np.float32(LAM) * (t_idx <= j_idx - 128 * r)
    tri_t = tri_t.astype(NPBF)
    mask_t = np.zeros((P, 2, 2, 512), dtype=np.float32)
    for pat in range(2):
        for s_ in range(2):
            mask_t[:, pat, s_] = 1.0 * (t_idx <= j_idx - 128 * (2 * pat + s_))
    iota_t = np.broadcast_to(
        (np.arange(T, dtype=np.float32) + 1.0) * np.float32(LAM), (P, T)
    ).copy()
    lam_t = np.zeros((P, KSUB, 4), dtype=np.float32)
    for kb in range(KSUB):
        for qtc in range(4):
            if kb < 4 * qtc:
                lam_t[:, kb, qtc] = np.float32(LAM)
    lam_t = lam_t.astype(NPBF)

    in_maps = []
    for c in range(N_CORES):
        b, h = c // 2, c % 2
        qrows = slice(h * EQ, (h + 1) * EQ)
        krows = slice(Q_DIM + h * EKV, Q_DIM + (h + 1) * EKV)
        vrows = slice(Q_DIM + KV_DIM + h * EKV, Q_DIM + (h + 1) * EKV + KV_DIM)
        wq_r = _re3(np.ascontiguousarray(w_qkv[qrows].T * WSCALE))
        wq_r4 = np.ascontiguousarray(
            wq_r.reshape(P, KSUB, H_Q, P).transpose(2, 0, 1, 3)
        ).astype(NPF8)  # [H_Q, P, 16, 128]
        xt = np.ascontiguousarray(x[b].T)
        in_maps.append(
            {
                "x16t": xt.astype(NPBF),
                "x8t": xt.astype(NPF8),
                "wq8": wq_r4,
                "wk8": _re3(np.ascontiguousarray(w_qkv[krows].T * WSCALE)).astype(
                    NPF8
                ),
                "wv16": _re3(np.ascontiguousarray(w_qkv[vrows].T)).astype(NPBF),
                "wo16": _re3(
                    np.ascontiguousarray(w_o[:, h * EQ : (h + 1) * EQ].T)
                ).reshape(P, H_Q, D).astype(NPBF),
                "cos_t": cos_t,
                "sin_t": sin_t,
                "ones_m": ones_m,
                "pswap": pswap,
                "ones8": ones8,
                "tri_t": tri_t,
                "mask_t": mask_t,
                "iota_t": iota_t,
                "lam_t": lam_t,
            }
        )
    return in_maps


def _gather(results):
    out = np.empty((B, T, D), dtype=np.float32)
    for b in range(B):
        acc = results[2 * b]["out_t"] + results[2 * b + 1]["out_t"]
        out[b] = acc.T
    return out


_NC_CACHE = []


def _get_module():
    if not _NC_CACHE:
        _NC_CACHE.append(_build_module())
    return _NC_CACHE[0]


def kernel(x, w_qkv, w_o):
    nc = _get_module()
    in_maps = _host_inputs(x, w_qkv, w_o)
    results = bass2jax.run_bass_via_pjrt(nc, in_maps, n_cores=N_CORES)
    return _gather(results)


# revision 5
# speedup vs baseline: 1.2321x; 1.2321x over previous
"""Causal GQA attention block (QK L2-norm + RoPE) for 8 trn2 NeuronCores.

Sharding: tensor-parallel over head-halves (2) x data-parallel over batch (4).
Core c handles batch c//2 and heads [h*8, h*8+8) with h = c%2.

Fast-path design (vs the fp32r baseline):
  - QK projection runs in fp8e4m3 with DoubleRow perf mode (256-deep
    contraction per pass = 2x PE throughput).  Weights are pre-scaled by
    64 so fp8 covers their range; the L2 norm absorbs the scale exactly.
  - Linear softmax: with QK-norm the logits are bounded by +-0.0884, so
    exp(p) is replaced by 1 + p (error ~p^2/2 < 0.4%).  att = 1 + SCALE*s
    splits attention into an exact "past-sum" base (bf16 matmuls against
    host triangle tables + per-qtile column sums of V) plus a correction
    d8 (x) v8 where d8 = fp8(raw scores) - both the correction AV and the
    softmax row-sum run as fp8 DoubleRow matmuls at 2x rate.
  - V path / scores / w_o in bf16; Q stays SBUF-resident (no DRAM spill).
  - Finalize is one fused DVE op: o = (o_ps + Sv) * recip(iota + sum).
All scale factors are folded into host tables (lambda = 1/SCALE).
"""

import numpy as np
import ml_dtypes

import concourse.mybir as mybir
import concourse.tile as tile
from concourse import bacc
from concourse import bass2jax

F32 = mybir.dt.float32
F32R = mybir.dt.float32r
BF16 = mybir.dt.bfloat16
F8 = mybir.dt.float8e4
AF = mybir.ActivationFunctionType
ALU = mybir.AluOpType
PM = mybir.MatmulPerfMode

NPF8 = ml_dtypes.float8_e4m3
NPBF = ml_dtypes.bfloat16

P = 128
B, T, D = 4, 2048, 2048
N_HEADS, HEAD_DIM, N_KV = 16, 128, 4
Q_DIM = N_HEADS * HEAD_DIM          # 2048
KV_DIM = N_KV * HEAD_DIM            # 512
H_Q = 8                             # q heads per core
H_KV = 2                            # kv heads per core
EQ = H_Q * HEAD_DIM                 # 1024 q features per core
EKV = H_KV * HEAD_DIM               # 256
SCALE = 0.08838834764831845
LAM = 1.0 / SCALE
WSCALE = 64.0                       # fp8 pre-scale on w_qk (norm absorbs it)
THETA = 10000.0

KSUB = D // P                       # 16 contraction subtiles
N_CORES = 8
TT_HALF = T // 2                    # 1024, phase-1 token half
NT512 = T // 512                    # 4 512-token q tiles
NTB = T // P                        # 16 128-token blocks


def _build_module():
    nc = bacc.Bacc("TRN2", target_bir_lowering=False, debug=False)

    x16t = nc.dram_tensor("x16t", [D, T], BF16, kind="ExternalInput")
    x8t = nc.dram_tensor("x8t", [D, T], F8, kind="ExternalInput")
    wq8 = nc.dram_tensor("wq8", [H_Q, P, KSUB, P], F8, kind="ExternalInput")
    wk8 = nc.dram_tensor("wk8", [P, KSUB, EKV], F8, kind="ExternalInput")
    wv16 = nc.dram_tensor("wv16", [P, KSUB, EKV], BF16, kind="ExternalInput")
    wo16 = nc.dram_tensor("wo16", [P, H_Q, D], BF16, kind="ExternalInput")
    cos_t = nc.dram_tensor("cos_t", [P, T], BF16, kind="ExternalInput")
    sin_t = nc.dram_tensor("sin_t", [P, T], BF16, kind="ExternalInput")
    ones_m = nc.dram_tensor("ones_m", [P, P], F32R, kind="ExternalInput")
    pswap = nc.dram_tensor("pswap", [P, P], F32R, kind="ExternalInput")
    ones8 = nc.dram_tensor("ones8", [P, 2, P], F8, kind="ExternalInput")
    tri_t = nc.dram_tensor("tri_t", [P, 4, 512], BF16, kind="ExternalInput")
    mask_t = nc.dram_tensor("mask_t", [P, 2, 2, 512], F32R, kind="ExternalInput")
    iota_t = nc.dram_tensor("iota_t", [P, T], F32, kind="ExternalInput")
    lam_t = nc.dram_tensor("lam_t", [P, KSUB, 4], BF16, kind="ExternalInput")
    out_t = nc.dram_tensor("out_t", [D, T], BF16, kind="ExternalOutput")

    with tile.TileContext(nc) as tc:
        with (
            tc.tile_pool(name="persist", bufs=1) as persist,
            tc.tile_pool(name="kv_persist", bufs=1) as kvp,
            tc.tile_pool(name="att_sb", bufs=5) as att_sb,
        ):
            ones_sb = persist.tile([P, P], F32R)
            psw_sb = persist.tile([P, P], F32R)
            ones8_sb = persist.tile([P, 2, P], F8)
            tri_sb = persist.tile([P, 4, 512], BF16)
            mask_sb = persist.tile([P, 2, 2, 512], F32R)
            iota_sb = persist.tile([P, T], F32)
            lam_sb = persist.tile([P, KSUB, 4], BF16)
            sv_sb = persist.tile([P, 2, 4], F32R)
            nc.sync.dma_start(ones_sb[:], ones_m.ap())
            nc.sync.dma_start(psw_sb[:], pswap.ap())
            nc.sync.dma_start(ones8_sb[:], ones8.ap())
            nc.gpsimd.dma_start(tri_sb[:], tri_t.ap())
            nc.gpsimd.dma_start(mask_sb[:], mask_t.ap())
            nc.gpsimd.dma_start(iota_sb[:], iota_t.ap())
            nc.gpsimd.dma_start(lam_sb[:], lam_t.ap())
            k_sb = kvp.tile([P, H_KV, T], BF16)     # roped+normed K^T slabs
            v16_sb = kvp.tile([P, NTB, EKV], BF16)  # V in [t, e] layout
            v8_sb = kvp.tile([P, NTB, EKV], F8)
            q_all = kvp.tile([P, H_Q, T], BF16)     # Q resident in SBUF

            # ---------------- phase 1: qkv proj + L2 norm + rope ----------
            with (
                tc.tile_pool(name="xres", bufs=1) as xres,
                tc.tile_pool(name="wstream", bufs=2) as wstream,
                tc.tile_pool(name="wvres", bufs=1) as wvres,
                tc.tile_pool(name="p1tmp", bufs=2) as p1tmp,
                tc.tile_pool(name="trig", bufs=1) as trig,
                tc.tile_pool(name="pp", bufs=2, space="PSUM") as pp,
                tc.tile_pool(name="pssq", bufs=2, space="PSUM") as pssq,
                tc.tile_pool(name="psw", bufs=2, space="PSUM") as psw,
                tc.tile_pool(name="pv", bufs=2, space="PSUM") as pv,
            ):
                cos_sb = trig.tile([P, T], BF16)
                sin_sb = trig.tile([P, T], BF16)
                wv_sb = wvres.tile([P, KSUB, EKV], BF16)
                wk_sb = wvres.tile([P, KSUB, EKV], F8, name="wk_sb")
                nc.sync.dma_start(wk_sb[:], wk8.ap())
                for th in range(2):
                    t0 = th * TT_HALF
                    x16_sb = xres.tile([P, KSUB, TT_HALF], BF16, tag="x16")
                    x8_sb = xres.tile([P, KSUB, TT_HALF], F8, tag="x8")
                    xr16 = x16t.ap()[:, t0 : t0 + TT_HALF].rearrange(
                        "(ks p) t -> p ks t", p=P
                    )
                    xr8 = x8t.ap()[:, t0 : t0 + TT_HALF].rearrange(
                        "(ks p) t -> p ks t", p=P
                    )
                    for ks in range(KSUB):
                        nc.sync.dma_start(x8_sb[:, ks], xr8[:, ks])
                    for ks in range(KSUB):
                        nc.sync.dma_start(x16_sb[:, ks], xr16[:, ks])
                    if th == 0:
                        nc.sync.dma_start(cos_sb[:], cos_t.ap())
                        nc.sync.dma_start(sin_sb[:], sin_t.ap())
                        nc.sync.dma_start(wv_sb[:], wv16.ap())

                    def proj_norm_rope(es):
                        """project feature block es (fp8 DoubleRow), norm, rope"""
                        if es < H_Q:
                            w_sb = wstream.tile([P, KSUB, P], F8, tag="w")
                            nc.sync.dma_start(w_sb[:], wq8.ap()[es])
                            w_use = w_sb
                        else:
                            w_use = wk_sb
                        for tt in range(2):
                            tg = t0 + tt * 512
                            sl = slice(tt * 512, (tt + 1) * 512)
                            raw_ps = pp.tile([P, 512], F32, tag="raw")
                            for kp in range(KSUB // 2):
                                if es < H_Q:
                                    lhs = w_use[:, 2 * kp : 2 * kp + 2, :]
                                else:
                                    e0 = (es - H_Q) * P
                                    lhs = w_use[:, 2 * kp : 2 * kp + 2, e0 : e0 + P]
                                nc.tensor.matmul(
                                    raw_ps[:],
                                    lhs,
                                    x8_sb[:, 2 * kp : 2 * kp + 2, sl],
                                    start=(kp == 0),
                                    stop=(kp == KSUB // 2 - 1),
                                    perf_mode=PM.DoubleRow,
                                )
                            sq = p1tmp.tile([P, 512], F32R, tag="t1")
                            nc.scalar.activation(sq[:], raw_ps[:], AF.Square)
                            ssq_ps = pssq.tile([P, 512], F32, tag="ssq")
                            nc.tensor.matmul(
                                ssq_ps[:], ones_sb[:], sq[:], start=True, stop=True
                            )
                            s_sb = p1tmp.tile([P, 512], F32, tag="t2")
                            nc.scalar.activation(s_sb[:], ssq_ps[:], AF.Sqrt)
                            r_sb = p1tmp.tile([P, 512], F32, tag="t3")
                            nc.vector.reciprocal_approx_fast(r_sb[:], s_sb[:])
                            qn = p1tmp.tile([P, 512], F32R, tag="t4")
                            nc.vector.tensor_mul(qn[:], raw_ps[:], r_sb[:])
                            ys = p1tmp.tile([P, 512], F32R, tag="t1")
                            nc.gpsimd.tensor_mul(
                                ys[:], qn[:], sin_sb[:, tg : tg + 512]
                            )
                            sw_ps = psw.tile([P, 512], F32, tag="sw")
                            nc.tensor.matmul(
                                sw_ps[:], psw_sb[:], ys[:], start=True, stop=True
                            )
                            qc = p1tmp.tile([P, 512], F32, tag="t2")
                            nc.gpsimd.tensor_mul(
                                qc[:], qn[:], cos_sb[:, tg : tg + 512]
                            )
                            if es < H_Q:
                                nc.vector.tensor_add(
                                    q_all[:, es, tg : tg + 512], sw_ps[:], qc[:]
                                )
                            else:
                                nc.vector.tensor_add(
                                    k_sb[:, es - H_Q, tg : tg + 512],
                                    sw_ps[:],
                                    qc[:],
                                )

                    # K first so attention can start earliest, then Q, then V
                    for es in (H_Q, H_Q + 1):
                        proj_norm_rope(es)
                    for es in range(H_Q):
                        proj_norm_rope(es)
                    for tb in range(TT_HALF // P):
                        tbg = th * (TT_HALF // P) + tb
                        v_ps = pv.tile([P, EKV], F32, tag="vp")
                        for ks in range(KSUB):
                            nc.tensor.matmul(
                                v_ps[:],
                                x16_sb[:, ks, tb * P : (tb + 1) * P],
                                wv_sb[:, ks],
                                start=(ks == 0),
                                stop=(ks == KSUB - 1),
                            )
                        nc.scalar.copy(v16_sb[:, tbg], v_ps[:])
                        nc.scalar.copy(v8_sb[:, tbg], v_ps[:])

            # ------- phase 2: attention + output projection per q-tile ----
            with (
                tc.tile_pool(name="wores", bufs=1) as wores,
                tc.tile_pool(name="p2tmp", bufs=2) as p2tmp,
                tc.tile_pool(name="oall", bufs=2) as oall,
                tc.tile_pool(name="fout", bufs=3) as fout,
                tc.tile_pool(name="psc", bufs=2, space="PSUM") as psc,
                tc.tile_pool(name="pav", bufs=2, space="PSUM") as pav,
                tc.tile_pool(name="psum2", bufs=1, space="PSUM") as psum2,
                tc.tile_pool(name="pf", bufs=1, space="PSUM") as pf,
            ):
                wo_sb = wores.tile([P, H_Q, D], BF16)
                for ei in range(H_Q):
                    nc.gpsimd.dma_start(wo_sb[:, ei], wo16.ap()[:, ei])
                # cumulative column sums of V (lambda-scaled) per (kv, qt)
                for kvi in range(H_KV):
                    sv_ps = pav.tile([P, 4], F32, tag="av")
                    for kb in range(NTB):
                        nc.tensor.matmul(
                            sv_ps[:],
                            v16_sb[:, kb, kvi * P : (kvi + 1) * P],
                            lam_sb[:, kb],
                            start=(kb == 0),
                            stop=(kb == NTB - 1),
                        )
                    nc.scalar.copy(sv_sb[:, kvi], sv_ps[:])
                for qt in range(NT512):
                    q0 = qt * 512
                    nkb = (qt + 1) * 4
                    npair = nkb // 2
                    o_all = oall.tile([P, H_Q, 512], BF16, tag="oa")
                    for hd in range(H_Q):
                        kvi = hd // 4
                        # per pair: scores -> d8 cast -> AV/sum accum, so d8
                        # tiles are consumed immediately (short lifetimes)
                        o_ps = pav.tile([P, 512], F32, tag="av")
                        s_ps = psum2.tile([P, 512], F32, tag="sum")
                        for pj in range(npair):
                            kb0 = 2 * pj
                            sc_ps = psc.tile([P, 2, 512], F32, tag="sc")
                            for j in range(2):
                                kb = kb0 + j
                                nc.tensor.matmul(
                                    sc_ps[:, j],
                                    k_sb[:, kvi, kb * P : (kb + 1) * P],
                                    q_all[:, hd, q0 : q0 + 512],
                                    start=True,
                                    stop=True,
                                )
                            d8 = att_sb.tile([P, 2, 512], F8, tag="att")
                            if pj >= npair - 2:
                                # diagonal pair: mask future keys in the cast
                                pat = pj - (npair - 2)
                                nc.vector.tensor_mul(
                                    d8[:], sc_ps[:], mask_sb[:, pat]
                                )
                            else:
                                nc.scalar.activation(d8[:], sc_ps[:], AF.Copy)
                            nc.tensor.matmul(
                                o_ps[:],
                                v8_sb[:, kb0 : kb0 + 2, kvi * P : (kvi + 1) * P],
                                d8[:],
                                start=(pj == 0),
                                stop=False,
                                perf_mode=PM.DoubleRow,
                                skip_group_check=True,
                            )
                            nc.tensor.matmul(
                                s_ps[:],
                                ones8_sb[:],
                                d8[:],
                                start=(pj == 0),
                                stop=(pj == npair - 1),
                                perf_mode=PM.DoubleRow,
                            )
                        for r in range(4):
                            kb = nkb - 4 + r
                            nc.tensor.matmul(
                                o_ps[:, 128 * r :],
                                v16_sb[:, kb, kvi * P : (kvi + 1) * P],
                                tri_sb[:, r, 128 * r :],
                                start=False,
                                stop=(r == 3),
                                skip_group_check=True,
                            )
                        den = p2tmp.tile([P, 512], F32, tag="den")
                        nc.vector.tensor_add(
                            den[:], s_ps[:], iota_sb[:, q0 : q0 + 512]
                        )
                        rs = p2tmp.tile([P, 512], F32, tag="rs")
                        nc.vector.reciprocal_approx_fast(rs[:], den[:])
                        nc.vector.scalar_tensor_tensor(
                            o_all[:, hd],
                            o_ps[:],
                            sv_sb[:, kvi, qt : qt + 1],
                            rs[:],
                            op0=ALU.add,
                            op1=ALU.mult,
                        )
                    for eo in range(D // P):
                        f_ps = pf.tile([P, 512], F32, tag="f")
                        for ei in range(H_Q):
                            nc.tensor.matmul(
                                f_ps[:],
                                wo_sb[:, ei, eo * P : (eo + 1) * P],
                                o_all[:, ei],
                                start=(ei == 0),
                                stop=(ei == H_Q - 1),
                            )
                        f_sb = fout.tile([P, 512], BF16, tag="fo")
                        nc.scalar.copy(f_sb[:], f_ps[:])
                        nc.sync.dma_start(
                            out_t.ap()[eo * P : (eo + 1) * P, q0 : q0 + 512],
                            f_sb[:],
                        )

    nc.compile()
    return nc


def _re3(a):
    """[K, E] -> [P, K//P, E] host rearrange for contiguous weight DMAs."""
    return np.ascontiguousarray(a.reshape(-1, P, a.shape[1]).transpose(1, 0, 2))


def _host_inputs(x, w_qkv, w_o):
    """Build the 8 per-core input maps from full inputs."""
    x = np.asarray(x, dtype=np.float32)
    w_qkv = np.asarray(w_qkv, dtype=np.float32)
    w_o = np.asarray(w_o, dtype=np.float32)

    half = HEAD_DIM // 2
    inv_freq = 1.0 / (
        THETA ** (np.arange(0, HEAD_DIM, 2, dtype=np.float32) / HEAD_DIM)
    )
    ang = np.arange(T, dtype=np.float32)[:, None] * inv_freq[None, :]  # [T, 64]
    cos = np.cos(ang).T.astype(np.float32)  # [64, T]
    sin = np.sin(ang).T.astype(np.float32)
    cos_t = np.ascontiguousarray(np.concatenate([cos, cos], axis=0)).astype(NPBF)
    sin_t = np.ascontiguousarray(np.concatenate([sin, sin], axis=0)).astype(NPBF)

    ones_m = np.ones((P, P), dtype=np.float32)
    pswap = np.zeros((P, P), dtype=np.float32)
    for p in range(half):
        pswap[p, p + half] = 1.0    # out[m=p+64] += ys[p]
        pswap[p + half, p] = -1.0   # out[m=p]    -= ys[p+64]
    ones8 = np.ones((P, 2, P), dtype=np.float32).astype(NPF8)

    t_idx = np.arange(P, dtype=np.float32)[:, None]        # key within block
    j_idx = np.arange(512, dtype=np.float32)[None, :]      # query col
    tri_t = np.zeros((P, 4, 512), dtype=np.float32)
    for r in range(4):
        tri_t[:, r] = np.float32(LAM) * (t_idx <= j_idx - 128 * r)
    tri_t = tri_t.astype(NPBF)
    mask_t = np.zeros((P, 2, 2, 512), dtype=np.float32)
    for pat in range(2):
        for s_ in range(2):
            mask_t[:, pat, s_] = 1.0 * (t_idx <= j_idx - 128 * (2 * pat + s_))
    iota_t = np.broadcast_to(
        (np.arange(T, dtype=np.float32) + 1.0) * np.float32(LAM), (P, T)
    ).copy()
    lam_t = np.zeros((P, KSUB, 4), dtype=np.float32)
    for kb in range(KSUB):
        for qtc in range(4):
            if kb < 4 * qtc:
                lam_t[:, kb, qtc] = np.float32(LAM)
    lam_t = lam_t.astype(NPBF)

    in_maps = []
    for c in range(N_CORES):
        b, h = c // 2, c % 2
        qrows = slice(h * EQ, (h + 1) * EQ)
        krows = slice(Q_DIM + h * EKV, Q_DIM + (h + 1) * EKV)
        vrows = slice(Q_DIM + KV_DIM + h * EKV, Q_DIM + (h + 1) * EKV + KV_DIM)
        wq_r = _re3(np.ascontiguousarray(w_qkv[qrows].T * WSCALE))
        wq_r4 = np.ascontiguousarray(
            wq_r.reshape(P, KSUB, H_Q, P).transpose(2, 0, 1, 3)
        ).astype(NPF8)  # [H_Q, P, 16, 128]
        xt = np.ascontiguousarray(x[b].T)
        in_maps.append(
            {
                "x16t": xt.astype(NPBF),
                "x8t": xt.astype(NPF8),
                "wq8": wq_r4,
                "wk8": _re3(np.ascontiguousarray(w_qkv[krows].T * WSCALE)).astype(
                    NPF8
                ),
                "wv16": _re3(np.ascontiguousarray(w_qkv[vrows].T)).astype(NPBF),
                "wo16": _re3(
                    np.ascontiguousarray(w_o[:, h * EQ : (h + 1) * EQ].T)
                ).reshape(P, H_Q, D).astype(NPBF),
                "cos_t": cos_t,
                "sin_t": sin_t,
                "ones_m": ones_m,
                "pswap": pswap,
                "ones8": ones8,
                "tri_t": tri_t,
                "mask_t": mask_t,
                "iota_t": iota_t,
                "lam_t": lam_t,
            }
        )
    return in_maps


def _gather(results):
    out = np.empty((B, T, D), dtype=np.float32)
    for b in range(B):
        acc = np.asarray(results[2 * b]["out_t"], np.float32) + np.asarray(
            results[2 * b + 1]["out_t"], np.float32
        )
        out[b] = acc.T
    return out


_NC_CACHE = []


def _get_module():
    if not _NC_CACHE:
        _NC_CACHE.append(_build_module())
    return _NC_CACHE[0]


def kernel(x, w_qkv, w_o):
    nc = _get_module()
    in_maps = _host_inputs(x, w_qkv, w_o)
    results = bass2jax.run_bass_via_pjrt(nc, in_maps, n_cores=N_CORES)
    return _gather(results)


# revision 7
# speedup vs baseline: 1.2378x; 1.0047x over previous
"""Causal GQA attention block (QK L2-norm + RoPE) for 8 trn2 NeuronCores.

Sharding: tensor-parallel over head-halves (2) x data-parallel over batch (4).
Core c handles batch c//2 and heads [h*8, h*8+8) with h = c%2.

Fast-path design (vs the fp32r baseline):
  - QK projection runs in fp8e4m3 with DoubleRow perf mode (256-deep
    contraction per pass = 2x PE throughput).  Weights are pre-scaled by
    64 so fp8 covers their range; the L2 norm absorbs the scale exactly.
  - Linear softmax: with QK-norm the logits are bounded by +-0.0884, so
    exp(p) is replaced by 1 + p (error ~p^2/2 < 0.4%).  att = 1 + SCALE*s
    splits attention into an exact "past-sum" base (bf16 matmuls against
    host triangle tables + per-qtile column sums of V) plus a correction
    d8 (x) v8 where d8 = fp8(raw scores) - both the correction AV and the
    softmax row-sum run as fp8 DoubleRow matmuls at 2x rate.
  - V path / scores / w_o in bf16; Q stays SBUF-resident (no DRAM spill).
  - Finalize is one fused DVE op: o = (o_ps + Sv) * recip(iota + sum).
All scale factors are folded into host tables (lambda = 1/SCALE).
"""

import numpy as np
import ml_dtypes

import concourse.mybir as mybir
import concourse.tile as tile
from concourse import bacc
from concourse import bass2jax

F32 = mybir.dt.float32
F32R = mybir.dt.float32r
BF16 = mybir.dt.bfloat16
F8 = mybir.dt.float8e4
AF = mybir.ActivationFunctionType
ALU = mybir.AluOpType
PM = mybir.MatmulPerfMode

NPF8 = ml_dtypes.float8_e4m3
NPBF = ml_dtypes.bfloat16

P = 128
B, T, D = 4, 2048, 2048
N_HEADS, HEAD_DIM, N_KV = 16, 128, 4
Q_DIM = N_HEADS * HEAD_DIM          # 2048
KV_DIM = N_KV * HEAD_DIM            # 512
H_Q = 8                             # q heads per core
H_KV = 2                            # kv heads per core
EQ = H_Q * HEAD_DIM                 # 1024 q features per core
EKV = H_KV * HEAD_DIM               # 256
SCALE = 0.08838834764831845
LAM = 1.0 / SCALE
WSCALE = 64.0                       # fp8 pre-scale on w_qk (norm absorbs it)
THETA = 10000.0

KSUB = D // P                       # 16 contraction subtiles
N_CORES = 8
TT_HALF = T // 2                    # 1024, phase-1 token half
NT512 = T // 512                    # 4 512-token q tiles
NTB = T // P                        # 16 128-token blocks


def _build_module():
    nc = bacc.Bacc("TRN2", target_bir_lowering=False, debug=False)

    x16t = nc.dram_tensor("x16t", [D, T], BF16, kind="ExternalInput")
    x8t = nc.dram_tensor("x8t", [D, T], F8, kind="ExternalInput")
    wq8 = nc.dram_tensor("wq8", [H_Q, P, KSUB, P], F8, kind="ExternalInput")
    wk8 = nc.dram_tensor("wk8", [P, KSUB, EKV], F8, kind="ExternalInput")
    wv16 = nc.dram_tensor("wv16", [P, KSUB, EKV], BF16, kind="ExternalInput")
    wo16 = nc.dram_tensor("wo16", [P, H_Q, D], BF16, kind="ExternalInput")
    cos_t = nc.dram_tensor("cos_t", [P, T], BF16, kind="ExternalInput")
    sin_t = nc.dram_tensor("sin_t", [P, T], BF16, kind="ExternalInput")
    ones_m = nc.dram_tensor("ones_m", [P, P], F32R, kind="ExternalInput")
    pswap = nc.dram_tensor("pswap", [P, P], F32R, kind="ExternalInput")
    ones8 = nc.dram_tensor("ones8", [P, 2, P], F8, kind="ExternalInput")
    tri_t = nc.dram_tensor("tri_t", [P, 4, 512], BF16, kind="ExternalInput")
    mask_t = nc.dram_tensor("mask_t", [P, 2, 2, 512], F32R, kind="ExternalInput")
    iota_t = nc.dram_tensor("iota_t", [P, T], F32, kind="ExternalInput")
    lam_t = nc.dram_tensor("lam_t", [P, KSUB, 4], BF16, kind="ExternalInput")
    out_t = nc.dram_tensor("out_t", [D, T], BF16, kind="ExternalOutput")

    with tile.TileContext(nc) as tc:
        with (
            tc.tile_pool(name="persist", bufs=1) as persist,
            tc.tile_pool(name="kv_persist", bufs=1) as kvp,
            tc.tile_pool(name="att_sb", bufs=5) as att_sb,
        ):
            ones_sb = persist.tile([P, P], F32R)
            psw_sb = persist.tile([P, P], F32R)
            ones8_sb = persist.tile([P, 2, P], F8)
            tri_sb = persist.tile([P, 4, 512], BF16)
            mask_sb = persist.tile([P, 2, 2, 512], F32R)
            iota_sb = persist.tile([P, T], F32)
            lam_sb = persist.tile([P, KSUB, 4], BF16)
            sv_sb = persist.tile([P, 2, 4], F32R)
            nc.sync.dma_start(ones_sb[:], ones_m.ap())
            nc.sync.dma_start(psw_sb[:], pswap.ap())
            nc.sync.dma_start(ones8_sb[:], ones8.ap())
            nc.gpsimd.dma_start(tri_sb[:], tri_t.ap())
            nc.gpsimd.dma_start(mask_sb[:], mask_t.ap())
            nc.gpsimd.dma_start(iota_sb[:], iota_t.ap())
            nc.gpsimd.dma_start(lam_sb[:], lam_t.ap())
            k_sb = kvp.tile([P, H_KV, T], BF16)     # roped+normed K^T slabs
            v16_sb = kvp.tile([P, NTB, EKV], BF16)  # V in [t, e] layout
            v8_sb = kvp.tile([P, NTB, EKV], F8)
            q_all = kvp.tile([P, H_Q, T], BF16)     # Q resident in SBUF

            # ---------------- phase 1: qkv proj + L2 norm + rope ----------
            with (
                tc.tile_pool(name="xres", bufs=1) as xres,
                tc.tile_pool(name="wstream", bufs=2) as wstream,
                tc.tile_pool(name="wvres", bufs=1) as wvres,
                tc.tile_pool(name="p1tmp", bufs=2) as p1tmp,
                tc.tile_pool(name="trig", bufs=1) as trig,
                tc.tile_pool(name="pp", bufs=2, space="PSUM") as pp,
                tc.tile_pool(name="pssq", bufs=2, space="PSUM") as pssq,
                tc.tile_pool(name="psw", bufs=2, space="PSUM") as psw,
                tc.tile_pool(name="pv", bufs=2, space="PSUM") as pv,
            ):
                cos_sb = trig.tile([P, T], BF16)
                sin_sb = trig.tile([P, T], BF16)
                wv_sb = wvres.tile([P, KSUB, EKV], BF16)
                wk_sb = wvres.tile([P, KSUB, EKV], F8, name="wk_sb")
                nc.scalar.dma_start(wk_sb[:], wk8.ap())
                for th in range(2):
                    t0 = th * TT_HALF
                    x16_sb = xres.tile([P, KSUB, TT_HALF], BF16, tag="x16")
                    x8_sb = xres.tile([P, KSUB, TT_HALF], F8, tag="x8")
                    xr16 = x16t.ap()[:, t0 : t0 + TT_HALF].rearrange(
                        "(ks p) t -> p ks t", p=P
                    )
                    xr8 = x8t.ap()[:, t0 : t0 + TT_HALF].rearrange(
                        "(ks p) t -> p ks t", p=P
                    )
                    for ks in range(KSUB):
                        eng = nc.sync if ks % 2 == 0 else nc.scalar
                        eng.dma_start(x8_sb[:, ks], xr8[:, ks])
                    for ks in range(KSUB):
                        nc.sync.dma_start(x16_sb[:, ks], xr16[:, ks])
                    if th == 0:
                        nc.sync.dma_start(cos_sb[:], cos_t.ap())
                        nc.sync.dma_start(sin_sb[:], sin_t.ap())
                        nc.sync.dma_start(wv_sb[:], wv16.ap())

                    def proj_norm_rope(es):
                        """project feature block es (fp8 DoubleRow), norm, rope"""
                        if es < H_Q:
                            w_sb = wstream.tile([P, KSUB, P], F8, tag="w")
                            nc.sync.dma_start(w_sb[:], wq8.ap()[es])
                            w_use = w_sb
                        else:
                            w_use = wk_sb
                        for tt in range(2):
                            tg = t0 + tt * 512
                            sl = slice(tt * 512, (tt + 1) * 512)
                            raw_ps = pp.tile([P, 512], F32, tag="raw")
                            for kp in range(KSUB // 2):
                                if es < H_Q:
                                    lhs = w_use[:, 2 * kp : 2 * kp + 2, :]
                                else:
                                    e0 = (es - H_Q) * P
                                    lhs = w_use[:, 2 * kp : 2 * kp + 2, e0 : e0 + P]
                                nc.tensor.matmul(
                                    raw_ps[:],
                                    lhs,
                                    x8_sb[:, 2 * kp : 2 * kp + 2, sl],
                                    start=(kp == 0),
                                    stop=(kp == KSUB // 2 - 1),
                                    perf_mode=PM.DoubleRow,
                                )
                            sq = p1tmp.tile([P, 512], F32R, tag="t1")
                            nc.scalar.activation(sq[:], raw_ps[:], AF.Square)
                            ssq_ps = pssq.tile([P, 512], F32, tag="ssq")
                            nc.tensor.matmul(
                                ssq_ps[:], ones_sb[:], sq[:], start=True, stop=True
                            )
                            s_sb = p1tmp.tile([P, 512], F32, tag="t2")
                            nc.scalar.activation(s_sb[:], ssq_ps[:], AF.Sqrt)
                            r_sb = p1tmp.tile([P, 512], F32, tag="t3")
                            nc.vector.reciprocal_approx_fast(r_sb[:], s_sb[:])
                            qn = p1tmp.tile([P, 512], F32R, tag="t4")
                            nc.vector.tensor_mul(qn[:], raw_ps[:], r_sb[:])
                            ys = p1tmp.tile([P, 512], F32R, tag="t1")
                            nc.gpsimd.tensor_mul(
                                ys[:], qn[:], sin_sb[:, tg : tg + 512]
                            )
                            sw_ps = psw.tile([P, 512], F32, tag="sw")
                            nc.tensor.matmul(
                                sw_ps[:], psw_sb[:], ys[:], start=True, stop=True
                            )
                            qc = p1tmp.tile([P, 512], F32, tag="t2")
                            nc.gpsimd.tensor_mul(
                                qc[:], qn[:], cos_sb[:, tg : tg + 512]
                            )
                            if es < H_Q:
                                nc.vector.tensor_add(
                                    q_all[:, es, tg : tg + 512], sw_ps[:], qc[:]
                                )
                            else:
                                nc.vector.tensor_add(
                                    k_sb[:, es - H_Q, tg : tg + 512],
                                    sw_ps[:],
                                    qc[:],
                                )

                    # K first so attention can start earliest, then Q, then V
                    for es in (H_Q, H_Q + 1):
                        proj_norm_rope(es)
                    for es in range(H_Q):
                        proj_norm_rope(es)
                    for tb in range(TT_HALF // P):
                        tbg = th * (TT_HALF // P) + tb
                        v_ps = pv.tile([P, EKV], F32, tag="vp")
                        for ks in range(KSUB):
                            nc.tensor.matmul(
                                v_ps[:],
                                x16_sb[:, ks, tb * P : (tb + 1) * P],
                                wv_sb[:, ks],
                                start=(ks == 0),
                                stop=(ks == KSUB - 1),
                            )
                        nc.scalar.copy(v16_sb[:, tbg], v_ps[:])
                        nc.scalar.copy(v8_sb[:, tbg], v_ps[:])

            # ------- phase 2: attention + output projection per q-tile ----
            with (
                tc.tile_pool(name="wores", bufs=1) as wores,
                tc.tile_pool(name="p2tmp", bufs=2) as p2tmp,
                tc.tile_pool(name="oall", bufs=2) as oall,
                tc.tile_pool(name="fout", bufs=3) as fout,
                tc.tile_pool(name="psc", bufs=2, space="PSUM") as psc,
                tc.tile_pool(name="pav", bufs=2, space="PSUM") as pav,
                tc.tile_pool(name="psum2", bufs=2, space="PSUM") as psum2,
            ):
                wo_sb = wores.tile([P, H_Q, D], BF16)
                for ei in range(H_Q):
                    nc.gpsimd.dma_start(wo_sb[:, ei], wo16.ap()[:, ei])
                # cumulative column sums of V (lambda-scaled) per (kv, qt)
                for kvi in range(H_KV):
                    sv_ps = pav.tile([P, 4], F32, tag="av")
                    for kb in range(NTB):
                        nc.tensor.matmul(
                            sv_ps[:],
                            v16_sb[:, kb, kvi * P : (kvi + 1) * P],
                            lam_sb[:, kb],
                            start=(kb == 0),
                            stop=(kb == NTB - 1),
                        )
                    nc.scalar.copy(sv_sb[:, kvi], sv_ps[:])
                for qt in range(NT512):
                    q0 = qt * 512
                    nkb = (qt + 1) * 4
                    npair = nkb // 2
                    o_all = oall.tile([P, H_Q, 512], BF16, tag="oa")
                    for hd in range(H_Q):
                        kvi = hd // 4
                        # per pair: scores -> d8 cast -> AV/sum accum, so d8
                        # tiles are consumed immediately (short lifetimes)
                        o_ps = pav.tile([P, 512], F32, tag="av")
                        s_ps = psum2.tile([P, 512], F32, tag="sum")
                        for pj in range(npair):
                            kb0 = 2 * pj
                            sc_ps = psc.tile([P, 2, 512], F32, tag="sc")
                            for j in range(2):
                                kb = kb0 + j
                                nc.tensor.matmul(
                                    sc_ps[:, j],
                                    k_sb[:, kvi, kb * P : (kb + 1) * P],
                                    q_all[:, hd, q0 : q0 + 512],
                                    start=True,
                                    stop=True,
                                )
                            d8 = att_sb.tile([P, 2, 512], F8, tag="att")
                            if pj >= npair - 2:
                                # diagonal pair: mask future keys in the cast
                                pat = pj - (npair - 2)
                                nc.vector.tensor_mul(
                                    d8[:], sc_ps[:], mask_sb[:, pat]
                                )
                            else:
                                nc.scalar.activation(d8[:], sc_ps[:], AF.Copy)
                            nc.tensor.matmul(
                                o_ps[:],
                                v8_sb[:, kb0 : kb0 + 2, kvi * P : (kvi + 1) * P],
                                d8[:],
                                start=(pj == 0),
                                stop=False,
                                perf_mode=PM.DoubleRow,
                                skip_group_check=True,
                            )
                            nc.tensor.matmul(
                                s_ps[:],
                                ones8_sb[:],
                                d8[:],
                                start=(pj == 0),
                                stop=(pj == npair - 1),
                                perf_mode=PM.DoubleRow,
                            )
                        for r in range(4):
                            kb = nkb - 4 + r
                            nc.tensor.matmul(
                                o_ps[:, 128 * r :],
                                v16_sb[:, kb, kvi * P : (kvi + 1) * P],
                                tri_sb[:, r, 128 * r :],
                                start=False,
                                stop=(r == 3),
                                skip_group_check=True,
                            )
                        den = p2tmp.tile([P, 512], F32, tag="den")
                        nc.vector.tensor_add(
                            den[:], s_ps[:], iota_sb[:, q0 : q0 + 512]
                        )
                        rs = p2tmp.tile([P, 512], F32, tag="rs")
                        nc.vector.reciprocal_approx_fast(rs[:], den[:])
                        nc.vector.scalar_tensor_tensor(
                            o_all[:, hd],
                            o_ps[:],
                            sv_sb[:, kvi, qt : qt + 1],
                            rs[:],
                            op0=ALU.add,
                            op1=ALU.mult,
                        )
                    for eo in range(D // P):
                        f_ps = psum2.tile([P, 512], F32, tag="sum")
                        for ei in range(H_Q):
                            nc.tensor.matmul(
                                f_ps[:],
                                wo_sb[:, ei, eo * P : (eo + 1) * P],
                                o_all[:, ei],
                                start=(ei == 0),
                                stop=(ei == H_Q - 1),
                            )
                        f_sb = fout.tile([P, 512], BF16, tag="fo")
                        nc.scalar.copy(f_sb[:], f_ps[:])
                        nc.sync.dma_start(
                            out_t.ap()[eo * P : (eo + 1) * P, q0 : q0 + 512],
                            f_sb[:],
                        )

    nc.compile()
    return nc


def _re3(a):
    """[K, E] -> [P, K//P, E] host rearrange for contiguous weight DMAs."""
    return np.ascontiguousarray(a.reshape(-1, P, a.shape[1]).transpose(1, 0, 2))


def _host_inputs(x, w_qkv, w_o):
    """Build the 8 per-core input maps from full inputs."""
    x = np.asarray(x, dtype=np.float32)
    w_qkv = np.asarray(w_qkv, dtype=np.float32)
    w_o = np.asarray(w_o, dtype=np.float32)

    half = HEAD_DIM // 2
    inv_freq = 1.0 / (
        THETA ** (np.arange(0, HEAD_DIM, 2, dtype=np.float32) / HEAD_DIM)
    )
    ang = np.arange(T, dtype=np.float32)[:, None] * inv_freq[None, :]  # [T, 64]
    cos = np.cos(ang).T.astype(np.float32)  # [64, T]
    sin = np.sin(ang).T.astype(np.float32)
    cos_t = np.ascontiguousarray(np.concatenate([cos, cos], axis=0)).astype(NPBF)
    sin_t = np.ascontiguousarray(np.concatenate([sin, sin], axis=0)).astype(NPBF)

    ones_m = np.ones((P, P), dtype=np.float32)
    pswap = np.zeros((P, P), dtype=np.float32)
    for p in range(half):
        pswap[p, p + half] = 1.0    # out[m=p+64] += ys[p]
        pswap[p + half, p] = -1.0   # out[m=p]    -= ys[p+64]
    ones8 = np.ones((P, 2, P), dtype=np.float32).astype(NPF8)

    t_idx = np.arange(P, dtype=np.float32)[:, None]        # key within block
    j_idx = np.arange(512, dtype=np.float32)[None, :]      # query col
    tri_t = np.zeros((P, 4, 512), dtype=np.float32)
    for r in range(4):
        tri_t[:, r] = np.float32(LAM) * (t_idx <= j_idx - 128 * r)
    tri_t = tri_t.astype(NPBF)
    mask_t = np.zeros((P, 2, 2, 512), dtype=np.float32)
    for pat in range(2):
        for s_ in range(2):
            mask_t[:, pat, s_] = 1.0 * (t_idx <= j_idx - 128 * (2 * pat + s_))
    iota_t = np.broadcast_to(
        (np.arange(T, dtype=np.float32) + 1.0) * np.float32(LAM), (P, T)
    ).copy()
    lam_t = np.zeros((P, KSUB, 4), dtype=np.float32)
    for kb in range(KSUB):
        for qtc in range(4):
            if kb < 4 * qtc:
                lam_t[:, kb, qtc] = np.float32(LAM)
    lam_t = lam_t.astype(NPBF)

    in_maps = []
    for c in range(N_CORES):
        b, h = c // 2, c % 2
        qrows = slice(h * EQ, (h + 1) * EQ)
        krows = slice(Q_DIM + h * EKV, Q_DIM + (h + 1) * EKV)
        vrows = slice(Q_DIM + KV_DIM + h * EKV, Q_DIM + (h + 1) * EKV + KV_DIM)
        wq_r = _re3(np.ascontiguousarray(w_qkv[qrows].T * WSCALE))
        wq_r4 = np.ascontiguousarray(
            wq_r.reshape(P, KSUB, H_Q, P).transpose(2, 0, 1, 3)
        ).astype(NPF8)  # [H_Q, P, 16, 128]
        xt = np.ascontiguousarray(x[b].T)
        in_maps.append(
            {
                "x16t": xt.astype(NPBF),
                "x8t": xt.astype(NPF8),
                "wq8": wq_r4,
                "wk8": _re3(np.ascontiguousarray(w_qkv[krows].T * WSCALE)).astype(
                    NPF8
                ),
                "wv16": _re3(np.ascontiguousarray(w_qkv[vrows].T)).astype(NPBF),
                "wo16": _re3(
                    np.ascontiguousarray(w_o[:, h * EQ : (h + 1) * EQ].T)
                ).reshape(P, H_Q, D).astype(NPBF),
                "cos_t": cos_t,
                "sin_t": sin_t,
                "ones_m": ones_m,
                "pswap": pswap,
                "ones8": ones8,
                "tri_t": tri_t,
                "mask_t": mask_t,
                "iota_t": iota_t,
                "lam_t": lam_t,
            }
        )
    return in_maps


def _gather(results):
    out = np.empty((B, T, D), dtype=np.float32)
    for b in range(B):
        acc = np.asarray(results[2 * b]["out_t"], np.float32) + np.asarray(
            results[2 * b + 1]["out_t"], np.float32
        )
        out[b] = acc.T
    return out


_NC_CACHE = []


def _get_module():
    if not _NC_CACHE:
        _NC_CACHE.append(_build_module())
    return _NC_CACHE[0]


def kernel(x, w_qkv, w_o):
    nc = _get_module()
    in_maps = _host_inputs(x, w_qkv, w_o)
    results = bass2jax.run_bass_via_pjrt(nc, in_maps, n_cores=N_CORES)
    return _gather(results)
